# revision 11
# baseline (speedup 1.0000x reference)
"""Trainium2 Bass kernel for a GRU "communication head".

Model (per reference):
    h0 = state @ Wp.T + bp                    (B, H)
    xs = embed[target]                        (B, T, E)
    for t: h = GRUCell(xs[:, t], h); logits_t = h @ Wo.T + bo
    out = stack logits                        (B, T, V)

Shapes: B=32, T=64, E=64, H=256, V=32003, INPUT_DIM=512.

Strategy (8 NeuronCores), cost-model driven:
  - Vocab(column)-parallel: V padded to 32768 = 8 * 4096; each core computes
    a 4096-row vocab slice of the logits for all (t, b). The small GRU
    recurrence is computed redundantly on every core.
  - Everything on the PE runs in bf16 (1 cyc/row at any N in the cost model,
    vs 4 cyc/row for small-N fp32r). Output is written to HBM in bf16
    (halves DMA bytes); the output bias bo is added on the HOST during
    assembly, along with the bf16->fp32 upconvert (tolerance is 2e-2).
  - Single 32-wide recurrence stream. Per step one PSUM bank holds all gate
    state [128, 14, 32]: slots 0-3 r/z pre-activations, 4-5 (W_hn h + b_hn),
    6-7 i_n, 8-9 r, 10-11 n-preact, 12-13 n. The x-side contributions
    (incl. all biases via a ones-row) are accumulated by h-independent
    matmuls emitted off the critical path.
  - Critical chain per step: W_hh MMs (r slots first) -> ACT sigmoid(r) ->
    DVE a=r*hn -> DVE b2=a+i_n -> ACT tanh -> DVE v=n*om -> DVE h=v+q
    (written directly as the bf16 master H). sigmoid(z), om=1-z (Pool) and
    q=z*h_prev (Pool) run off the critical path.
  - Output projection: bf16 MM pairs (ki=0,1 accumulate) paced ~2 units per
    step so the PE fills the chain's latency windows and stays p-state-warm
    (gaps < ~4us never reset warmth). PSUM->SBUF bf16 copies alternate
    between ACT and DVE in the chain's idle slots; 1MB grouped DMAs.
    The last 512 columns use narrow blocks (256/128) so the post-step-63
    tail stays small.
"""

import numpy as np
import ml_dtypes

B = 32
T = 64
E = 64
H = 256
V = 32003
INPUT_DIM = 512
NCORES = 8
VPAD = 32768
VLOC = VPAD // NCORES  # 4096
TB = T * B  # 2048
KST = 640  # padded (INPUT_DIM + bias row) -> 5 chunks of 128
NM = VLOC // 128  # 32 vocab chunks per core

# (c0, ncols, ready_step): projection col-blocks; a block's MMs may be
# emitted from step ready_step+1 on (H columns complete after ready_step)
BLOCKS = [
    (0, 512, 15),
    (512, 512, 31),
    (1024, 512, 47),
    (1536, 256, 55),
    (1792, 128, 59),
    (1920, 128, 63),
]
# output DMA chunks: [c0, 512) fired per m-group of 8 when fully staged
DMA_CHUNKS = [0, 512, 1024, 1536]

_CACHE = {}


def _build_nc(debug_h=False):
    import concourse.mybir as mybir
    import concourse.tile as tile
    from concourse import bacc

    f32 = mybir.dt.float32
    bf16 = mybir.dt.bfloat16
    AF = mybir.ActivationFunctionType
    ALU = mybir.AluOpType

    nc = bacc.Bacc(
        "TRN2",
        debug=False,
        enable_asserts=False,
        target_bir_lowering=False,
        num_devices=NCORES,
    )

    d_xsT = nc.dram_tensor("xsT", (128, TB), bf16, kind="ExternalInput")
    d_wihT = nc.dram_tensor("wihT", (128, 3 * H), bf16, kind="ExternalInput")
    d_whhT = nc.dram_tensor("whhT", (H, 3 * H), bf16, kind="ExternalInput")
    d_bhhn = nc.dram_tensor("bhhn", (1, H), bf16, kind="ExternalInput")
    d_stT = nc.dram_tensor("stT", (KST, B), bf16, kind="ExternalInput")
    d_wpT = nc.dram_tensor("wpT", (KST, H), bf16, kind="ExternalInput")
    d_woT = nc.dram_tensor("woT", (H, VLOC), bf16, kind="ExternalInput")
    # vocab-major output: row v (local, = m*128 + partition), col t*B+b
    d_out = nc.dram_tensor("out", (VLOC, TB), bf16, kind="ExternalOutput")
    d_dbgH = (
        nc.dram_tensor("dbgH", (128, 2, TB), bf16, kind="ExternalOutput")
        if debug_h
        else None
    )

    with tile.TileContext(nc) as tc:
        with (
            tc.tile_pool(name="weights", bufs=1) as wpool,
            tc.tile_pool(name="gates_ps", bufs=2, space="PSUM") as gps,
            tc.tile_pool(name="proj_ps", bufs=5, space="PSUM") as lps,
            tc.tile_pool(name="tmp", bufs=24) as tmp,
            tc.tile_pool(name="stage", bufs=8) as ost,
        ):
            # ---- persistent SBUF loads (small first so h0/steps start early)
            stT = wpool.tile([128, 5, B], bf16, tag="stT")
            nc.sync.dma_start(
                out=stT, in_=d_stT.ap().rearrange("(kc p) b -> p kc b", p=128)
            )
            wpT = wpool.tile([128, 5, H], bf16, tag="wpT")
            nc.sync.dma_start(
                out=wpT, in_=d_wpT.ap().rearrange("(kc p) m -> p kc m", p=128)
            )
            bhhn = wpool.tile([1, H], bf16, tag="bhhn")
            nc.gpsimd.dma_start(out=bhhn, in_=d_bhhn.ap())
            wihT = wpool.tile([128, 3 * H], bf16, tag="wihT")
            nc.sync.dma_start(out=wihT, in_=d_wihT.ap())
            whhT = wpool.tile([128, 2, 3 * H], bf16, tag="whhT")
            nc.sync.dma_start(
                out=whhT, in_=d_whhT.ap().rearrange("(kc p) m -> p kc m", p=128)
            )
            xsT = wpool.tile([128, TB], bf16, tag="xsT")
            nc.sync.dma_start(out=xsT, in_=d_xsT.ap())
            woT = wpool.tile([128, 2, VLOC], bf16, tag="woT")
            woT_src = d_woT.ap().rearrange("(kc p) v -> p kc v", p=128)
            for c2 in range(2):
                vs2 = slice(c2 * (VLOC // 2), (c2 + 1) * (VLOC // 2))
                nc.sync.dma_start(out=woT[:, :, vs2], in_=woT_src[:, :, vs2])

            ones = wpool.tile([1, B], bf16, tag="ones")
            nc.vector.memset(ones, 1.0)
            z128 = wpool.tile([1, 128], bf16, tag="z128")
            nc.vector.memset(z128, 0.0)
            ones256 = wpool.tile([1, 8 * B], bf16, tag="ones256")
            nc.vector.memset(ones256, 1.0)

            # bf16 master hidden state, col = t*B + b
            H_bf = wpool.tile([128, 2, TB], bf16, tag="H_bf")
            h0_bf = wpool.tile([128, 2, B], bf16, tag="h0_bf")

            out_g = d_out.ap().rearrange("(g p) t -> p g t", p=128)

            # ---- h0 = state @ Wp.T + bp (bias folded into padded row 512)
            ps0 = lps.tile([128, 512], f32, tag="lg", name="h0ps")
            for ko in range(2):
                for ki in range(5):
                    nc.tensor.matmul(
                        ps0[:, ko * B : (ko + 1) * B],
                        wpT[:, ki, ko * 128 : (ko + 1) * 128],
                        stT[:, ki, :],
                        start=(ki == 0),
                        stop=(ki == 4),
                        skip_group_check=True,
                    )
            nc.vector.tensor_copy(
                h0_bf.rearrange("p a b -> p (a b)"), ps0[:, 0 : 2 * B]
            )

            # ---- projection machinery ----
            # unit = (c0, ncols, m): 2 MMs (ki acc) -> [128, ncols] PSUM,
            # then one ACT-or-DVE copy into the bf16 stage tile, then a 1MB
            # DMA per (m-group, 512-col chunk) when fully staged.
            pend_mm = []     # units ready for MM emission
            pend_copy = []   # (ps, c0, ncols, m) awaiting copy emission
            stage_tiles = {}  # (g, chunk_idx) -> stage tile
            stage_count = {}  # (g, chunk_idx) -> copies landed
            stage_need = {}   # (g, chunk_idx) -> copies required
            for g in range(4):
                for ci, c0 in enumerate(DMA_CHUNKS):
                    need = sum(
                        1
                        for (b0, bn, _r) in BLOCKS
                        if c0 <= b0 < c0 + 512
                    ) * 8
                    stage_need[(g, ci)] = need
                    stage_count[(g, ci)] = 0

            copy_flip = [0]

            def emit_proj_mms(budget_ns):
                used = 0.0
                while pend_mm and used < budget_ns:
                    c0, ncols, m = pend_mm.pop(0)
                    psl = lps.tile([128, 512], f32, tag="lg", name="lg")
                    ps = psl[:, 0:ncols]
                    cs = slice(c0, c0 + ncols)
                    nc.tensor.matmul(
                        ps,
                        woT[:, 0, m * 128 : (m + 1) * 128],
                        H_bf[:, 0, cs],
                        start=True,
                        stop=False,
                    )
                    nc.tensor.matmul(
                        ps,
                        woT[:, 1, m * 128 : (m + 1) * 128],
                        H_bf[:, 1, cs],
                        start=False,
                        stop=True,
                    )
                    pend_copy.append((ps, c0, ncols, m))
                    used += 2 * (ncols * 0.4167 + 5)

            def emit_copies(engine, budget_ns):
                # engine: 0 = ACT, 1 = DVE
                used = 0.0
                fired = []
                while pend_copy and used < budget_ns:
                    ps, c0, ncols, m = pend_copy.pop(0)
                    g = m // 8
                    ci = c0 // 512
                    key = (g, ci)
                    if key not in stage_tiles:
                        stage_tiles[key] = ost.tile(
                            [128, 8, 512], mybir.dt.bfloat16, tag="ob", name="ob"
                        )
                    off = c0 - DMA_CHUNKS[ci]
                    dst = stage_tiles[key][:, m % 8, off : off + ncols]
                    if engine == 0:
                        nc.scalar.copy(dst, ps)
                        used += ncols * 0.833 + 190
                    else:
                        nc.vector.tensor_copy(dst, ps)
                        used += ncols * 1.042 + 130
                    stage_count[key] += 1
                    if stage_count[key] == stage_need[key]:
                        fired.append(key)
                for g, ci in fired:
                    c0 = DMA_CHUNKS[ci]
                    nc.sync.dma_start(
                        out=out_g[:, g * 8 : (g + 1) * 8, c0 : c0 + 512],
                        in_=stage_tiles.pop((g, ci)),
                    )

            # ---- recurrence ----
            # PSUM slot layout (dim1 of P, [128, 14, 32] f32):
            #   0-1 r-pre, 2-3 z-pre, 4-5 hn(+b_hn), 6-7 i_n,
            #   8-9 r, 10-11 n-pre (b2), 12-13 n
            for t in range(T):
                cs = slice(t * B, (t + 1) * B)
                hprev = h0_bf if t == 0 else H_bf[:, :, (t - 1) * B : t * B]

                P = gps.tile([128, 14, B], f32, tag="g", name="P")

                # x-side + biases: independent of h ----------------------
                # One zero-opener MM owns start=True for the whole bank
                # (slots 0-7); everything else accumulates, and only the
                # final W-MM of the step carries stop=True.
                nc.tensor.matmul(
                    P[:, 0:8, :],
                    z128,
                    ones256,
                    start=True,
                    stop=False,
                    skip_group_check=True,
                )
                for blk in range(6):
                    slot = blk if blk < 4 else 2 + blk  # 0-3 rz, 6-7 i_n
                    nc.tensor.matmul(
                        P[:, slot, :],
                        wihT[:, blk * 128 : (blk + 1) * 128],
                        xsT[:, cs],
                        start=False,
                        stop=False,
                        skip_group_check=True,
                    )
                for ki in range(2):
                    nc.tensor.matmul(
                        P[:, 4 + ki, :],
                        bhhn[0:1, ki * 128 : (ki + 1) * 128],
                        ones,
                        start=False,
                        stop=False,
                        skip_group_check=True,
                    )
                # h-side: r slots first so sigmoid(r) unblocks earliest ---
                for blk in (0, 1, 2, 3, 4, 5):
                    slot = blk
                    for ki in range(2):
                        nc.tensor.matmul(
                            P[:, slot, :],
                            whhT[:, ki, blk * 128 : (blk + 1) * 128],
                            hprev[:, ki, :],
                            start=False,
                            stop=(blk == 5 and ki == 1),
                            skip_group_check=True,
                        )
                # projection MMs fill the chain's PE idle window ---------
                for b0, bn, rs in BLOCKS:
                    if rs == t - 1:
                        pend_mm.extend((b0, bn, m) for m in range(NM))
                emit_proj_mms(900 if t < 56 else 1400)

                # ---- gate math ----
                r_sb = tmp.tile([128, 2, B], f32, tag="r", name="r")
                z_sb = tmp.tile([128, 2, B], f32, tag="z", name="z")
                a_sb = tmp.tile([128, 2, B], f32, tag="a", name="a")
                om_sb = tmp.tile([128, 2, B], f32, tag="om", name="om")
                q_sb = tmp.tile([128, 2, B], f32, tag="q", name="q")
                v_sb = tmp.tile([128, 2, B], f32, tag="v", name="v")

                nc.scalar.activation(out=r_sb, in_=P[:, 0:2, :], func=AF.Sigmoid)
                nc.scalar.activation(out=z_sb, in_=P[:, 2:4, :], func=AF.Sigmoid)
                emit_copies(0, 1200)  # ACT copies run during the DVE phase
                nc.vector.tensor_mul(a_sb, r_sb, P[:, 4:6, :])
                nc.vector.tensor_add(P[:, 10:12, :], a_sb, P[:, 6:8, :])
                emit_copies(1, 1200)  # DVE copies run during the tanh phase
                nc.gpsimd.tensor_scalar(
                    om_sb, z_sb, -1.0, 1.0, ALU.mult, ALU.add
                )
                nc.gpsimd.tensor_mul(q_sb, z_sb, hprev)
                nc.scalar.activation(
                    out=P[:, 12:14, :], in_=P[:, 10:12, :], func=AF.Tanh
                )
                nc.vector.tensor_mul(v_sb, P[:, 12:14, :], om_sb)
                nc.vector.tensor_add(H_bf[:, :, cs], v_sb, q_sb)

            if d_dbgH is not None:
                nc.sync.dma_start(out=d_dbgH.ap(), in_=H_bf)

            # ---- drain (blocks gated on the final step included) ----
            for b0, bn, rs in BLOCKS:
                if rs >= T - 1:
                    pend_mm.extend((b0, bn, m) for m in range(NM))
            while pend_mm or pend_copy:
                emit_proj_mms(3000)
                emit_copies(0, 1000)
                emit_copies(1, 1000)

    nc.compile()
    return nc


def _get_nc():
    if "nc" not in _CACHE:
        _CACHE["nc"] = _build_nc()
    return _CACHE["nc"]


def _prep_in_maps(state, target, embed, Wp, bp, W_ih, W_hh, b_ih, b_hh, Wo, bo):
    bf = ml_dtypes.bfloat16
    f = np.float32
    state = np.asarray(state, dtype=f)
    target = np.asarray(target)
    embed = np.asarray(embed, dtype=f)
    Wp = np.asarray(Wp, dtype=f)
    bp = np.asarray(bp, dtype=f)
    W_ih = np.asarray(W_ih, dtype=f)
    W_hh = np.asarray(W_hh, dtype=f)
    b_ih = np.asarray(b_ih, dtype=f)
    b_hh = np.asarray(b_hh, dtype=f)
    Wo = np.asarray(Wo, dtype=f)

    # host-side gather + transpose to (E, T*B), col = t*B + b
    xs = embed[target.astype(np.int64)]  # (B, T, E)
    xsT = np.ascontiguousarray(xs.transpose(1, 0, 2).reshape(TB, E).T)
    xsT_pad = np.zeros((128, TB), f)
    xsT_pad[:E] = xsT
    xsT_pad[E] = 1.0  # bias row

    bias_gi = np.concatenate([b_ih[: 2 * H] + b_hh[: 2 * H], b_ih[2 * H :]])
    wihT_pad = np.zeros((128, 3 * H), f)
    wihT_pad[:E] = W_ih.T
    wihT_pad[E] = bias_gi

    whhT = np.ascontiguousarray(W_hh.T)  # (H, 3H)
    bhhn = np.ascontiguousarray(b_hh[2 * H :][None, :])  # (1, H)

    stT_pad = np.zeros((KST, B), f)
    stT_pad[:INPUT_DIM] = state.T
    stT_pad[INPUT_DIM] = 1.0
    wpT_pad = np.zeros((KST, H), f)
    wpT_pad[:INPUT_DIM] = Wp.T
    wpT_pad[INPUT_DIM] = bp

    woT_full = np.zeros((H, VPAD), f)
    woT_full[:, :V] = Wo.T

    common = {
        "xsT": xsT_pad.astype(bf),
        "wihT": wihT_pad.astype(bf),
        "whhT": whhT.astype(bf),
        "bhhn": bhhn.astype(bf),
        "stT": stT_pad.astype(bf),
        "wpT": wpT_pad.astype(bf),
    }
    in_maps = []
    for c in range(NCORES):
        vs = slice(c * VLOC, (c + 1) * VLOC)
        m = dict(common)
        m["woT"] = np.ascontiguousarray(woT_full[:, vs]).astype(bf)
        in_maps.append(m)
    return in_maps


def _assemble(results, bo):
    full = np.concatenate([r["out"] for r in results], axis=0)  # (VPAD, TB) bf16
    # out[b, t, v] = full[v, t*B + b] + bo[v]
    out = full[:V].reshape(V, T, B).transpose(2, 1, 0).astype(np.float32)
    out += np.asarray(bo, dtype=np.float32)[None, None, :]
    return np.ascontiguousarray(out)


def _run(in_maps, **kwargs):
    from concourse.bass_utils import run_bass_kernel_spmd

    nc = _get_nc()
    return run_bass_kernel_spmd(nc, in_maps, core_ids=list(range(NCORES)), **kwargs)


def kernel(**inputs):
    in_maps = _prep_in_maps(**inputs)
    res = _run(in_maps)
    return _assemble(res.results, inputs["bo"])
